# revision 1
# baseline (speedup 1.0000x reference)
"""Self-contained Trainium2 Bass kernel for the "Attentive" GNN message-passing
problem:

    x: [8192, 256] f32, attn_vectors: [4, 256] f32
    e_h = l2_normalize(attn_vectors[h] * x, axis=-1)        # [H, N, D]
    Y   = concat_h(e_h)                                     # [N, H*D]
    out = (Y @ Y.T) / H                                     # [N, N]

Strategy (8 NeuronCores, SPMD, no collectives):
  - Output rows are sharded 8 x 1024; every core receives the FULL x plus its
    own x_local row-shard as separate inputs, so the program is core-agnostic.
  - Key algebra: out[i,j] = sum_k (x*a^2*r/H)[i,k] * (x*r)[j,k] with
    r_h[n] = 1/sqrt(max(sum_d (a_h[d]*x[n,d])^2, eps)); a^2 and the 1/H are
    folded into the (small, resident) lhsT side only, so the streamed rhs
    panels need just one elementwise multiply each.
  - Everything runs in "features on partitions" layout (x^T), obtained by
    staging a bf16 copy of x in DRAM (per-panel tiles, fine-grained deps)
    and reading it back through the DMA xbar transpose.
  - Row norms are computed as transposed PE matmuls xsq^T @ a^2 so the
    max/sqrt/reciprocal chain runs in [128, 16] layout (all DVE lanes
    active); rnorm rows bounce through DRAM and come back as one batched
    broadcast DMA per panel (step-0 partition APs are legal on DRAM).
  - Matmul inputs are bf16 (PE runs f32 at quarter rate); PSUM accumulates
    f32; each panel's 8 PSUM tiles are copied into one SBUF tile and leave
    in a single 2 MB DMA.
  - DMAs are batched aggressively: the SP sequencer pays ~600 ns per
    dma_start, so the panel pipeline uses ~7 DMAs per 512-column panel.
"""

from contextlib import ExitStack

import numpy as np

N, D, H = 8192, 256, 4
NCORES = 8
NLOC = N // NCORES  # 1024 output rows per core
P = 128
PANEL = 512
NPANELS = N // PANEL  # 16
RBLK = NLOC // P  # 8 row blocks of the local output
KCH = (H * D) // P  # 8 contraction chunks of 128
CHD = D // P  # 2 chunks per head
SUB = PANEL // P  # 4 column sub-blocks per panel
EPS = 1e-12

_COMPILED = {}


def _build_bass():
    import concourse.bass as bass
    import concourse.tile as tile
    from concourse import bacc, mybir

    f32 = mybir.dt.float32
    bf16 = mybir.dt.bfloat16

    nc = bacc.Bacc(
        "TRN2",
        target_bir_lowering=False,
        debug=False,
        enable_asserts=False,
        num_devices=NCORES,
    )
    x_t = nc.dram_tensor("x", [N, D], f32, kind="ExternalInput")
    xl_t = nc.dram_tensor("x_local", [NLOC, D], f32, kind="ExternalInput")
    # Host-precomputed functions of attn_vectors (tiny):
    #   w_sq[d, c*4+h]  = attn[h, c*128+d]^2          (bf16, norm matmul rhs)
    #   asq[d, kc]      = 0.25*attn[h, c*128+d]^2     (f32, kc = h*2+c)
    ws_t = nc.dram_tensor("w_sq", [P, CHD * H], bf16, kind="ExternalInput")
    aq_t = nc.dram_tensor("asq", [P, KCH], f32, kind="ExternalInput")
    out_t = nc.dram_tensor("out", [NLOC, N], f32, kind="ExternalOutput")

    x, xl, out = x_t.ap(), xl_t.ap(), out_t.ap()

    with tile.TileContext(nc) as tc, ExitStack() as ctx:
        consts = ctx.enter_context(tc.tile_pool(name="consts", bufs=1))
        loads = ctx.enter_context(tc.tile_pool(name="loads", bufs=6))
        xtp = ctx.enter_context(tc.tile_pool(name="xtp", bufs=1))
        sq = ctx.enter_context(tc.tile_pool(name="sq", bufs=4))
        small = ctx.enter_context(tc.tile_pool(name="small", bufs=3))
        bcp = ctx.enter_context(tc.tile_pool(name="bcp", bufs=3))
        rhsp = ctx.enter_context(tc.tile_pool(name="rhsp", bufs=3))
        outp = ctx.enter_context(tc.tile_pool(name="outp", bufs=2))
        dram = ctx.enter_context(tc.tile_pool(name="dram", bufs=1, space="DRAM"))
        ps_norm = ctx.enter_context(
            tc.tile_pool(name="ps_norm", bufs=2, space="PSUM")
        )
        ps_out = ctx.enter_context(
            tc.tile_pool(name="ps_out", bufs=4, space="PSUM")
        )

        from concourse.masks import make_identity

        w_sq = consts.tile([P, CHD * H], bf16)
        nc.sync.dma_start(w_sq[:], ws_t.ap()[:])
        asq = consts.tile([P, KCH], f32)
        nc.sync.dma_start(asq[:], aq_t.ap()[:])
        ident = consts.tile([P, P], f32)
        make_identity(nc, ident[:])
        identb = consts.tile([P, P], bf16)
        make_identity(nc, identb[:])

        def sb_rearr(tile_ap):
            return tile_ap[:].rearrange("q (i d) -> q i d", i=SUB)

        def x_rearr(ap, row0):
            return ap[row0 : row0 + PANEL, :].rearrange(
                "(i q) d -> q i d", q=P
            )

        def prepass(src_ap, row0, xT_tile, name):
            """Load 512 source rows (one batched DMA), transpose them on the
            PE into bf16 x^T, and park this panel's rnorm in DRAM.
            The PSUM->SBUF copy after each transpose doubles as the f32->bf16
            cast."""
            xt = loads.tile([P, SUB * D], f32, tag="xload")
            nc.sync.dma_start(sb_rearr(xt), x_rearr(src_ap, row0))
            # Round to bf16 before the PE transpose: bf16 streams the PE at
            # 1 cycle/row vs 2 for f32, and the rounding happens exactly once
            # either way (the PSUM->SBUF copy used to do it).
            xtb = loads.tile([P, SUB * D], bf16, tag="xtb")
            nc.vector.tensor_copy(xtb[:], xt[:])
            for c in range(CHD):
                tp4 = ps_norm.tile([P, PANEL], bf16, tag="tp")
                for i in range(SUB):
                    nc.tensor.transpose(
                        tp4[:, i * P : (i + 1) * P],
                        xtb[:, i * D + c * P : i * D + (c + 1) * P],
                        identb[:],
                    )
                nc.vector.tensor_copy(
                    xT_tile[:, c * PANEL : (c + 1) * PANEL], tp4[:]
                )
            pn = ps_norm.tile([P, SUB * H], f32, tag="pn")
            xsq = sq.tile([P, CHD * PANEL], bf16, tag="xsq")
            nc.vector.tensor_mul(xsq[:], xT_tile[:], xT_tile[:])
            for i in range(SUB):
                for c in range(CHD):
                    nc.tensor.matmul(
                        pn[:, i * H : (i + 1) * H],
                        xsq[:, c * PANEL + i * P : c * PANEL + (i + 1) * P],
                        w_sq[:, c * H : (c + 1) * H],
                        start=(c == 0),
                        stop=(c == CHD - 1),
                    )
            # eps-clamp; the input AP also permutes [q,(i h)] -> [q,(h i)]
            # so that after the PE transpose the store is contiguous.
            clamped = small.tile([P, SUB * H], f32, tag="clamped")
            nc.vector.tensor_scalar_max(
                clamped[:],
                pn[:].rearrange("q (i h) -> q h i", h=H),
                EPS,
            )
            root = small.tile([P, SUB * H], f32, tag="root")
            nc.scalar.sqrt(root[:], clamped[:])
            rnorm = small.tile([P, SUB * H], f32, tag="rnorm")
            nc.vector.reciprocal(rnorm[:], root[:])
            # [128, 16] -> [16, 128]; row j = h*4+i, so the flat DRAM tile
            # is rnorm_h[i*128+q] at offset h*512 + i*128 + q (h-major).
            pt = ps_norm.tile([SUB * H, P], f32, tag="tp")
            nc.tensor.transpose(pt[:], rnorm[:], ident[:])
            rno = small.tile([SUB * H, P], f32, tag="rno")
            nc.vector.tensor_copy(rno[:], pt[:])
            rnd = dram.tile([SUB * H, P], f32, name=name)
            nc.sync.dma_start(rnd[:], rno[:])
            return rnd

        def bcast_rnorm(rnd):
            """[128, 4*512] f32: bc[:, h*512 + n] = rnorm_h[n], one DMA."""
            bc = bcp.tile([P, H * PANEL], f32, tag="bc")
            src = bass.AP(
                rnd.tensor,
                rnd.offset,
                [[0, P], [PANEL, H], [1, PANEL]],
            )
            nc.sync.dma_start(
                bc[:].rearrange("p (h n) -> p h n", h=H), src
            )
            return bc

        # ---- all prepasses first ------------------------------------------
        # Tile's per-engine instruction order is static, so the lhsT-build
        # DVE ops (which wait on the rnorm DRAM bounce) must come AFTER every
        # prepass op or they head-of-line-block the prepass copies and starve
        # the PE of transpose work during the wait.
        lhsT = consts.tile([P, KCH * NLOC], bf16)
        xlocT = []
        lrnds = []
        for lp in range(2):
            t = consts.tile([P, CHD * PANEL], bf16, name=f"xlocT{lp}")
            xlocT.append(t)
            lrnds.append(prepass(xl, lp * PANEL, t, f"lrnd{lp}"))
        PIPE = 4  # panels of prepass lookahead over the main loop
        xTs = []
        rnds = []

        def prepass_x(p):
            t = xtp.tile([P, CHD * PANEL], bf16, name=f"xT{p}")
            xTs.append(t)
            rnds.append(prepass(x, p * PANEL, t, f"rnd{p}"))

        for p in range(PIPE):
            prepass_x(p)

        # ---- resident lhsT -------------------------------------------------
        for lp in range(2):
            t = xlocT[lp]
            bc = bcast_rnorm(lrnds[lp])
            for h in range(H):
                for c in range(CHD):
                    kc = h * CHD + c
                    scaled = sq.tile([P, PANEL], f32, tag="scaled")
                    nc.vector.tensor_scalar_mul(
                        scaled[:],
                        bc[:, h * PANEL : (h + 1) * PANEL],
                        asq[:, kc : kc + 1],
                    )
                    nc.vector.tensor_mul(
                        lhsT[
                            :,
                            kc * NLOC + lp * PANEL : kc * NLOC + (lp + 1) * PANEL,
                        ],
                        t[:, c * PANEL : (c + 1) * PANEL],
                        scaled[:],
                    )

        # ---- main loop over 16 column panels (prepass pipelined ahead) -----
        for p in range(NPANELS):
            bc = bcast_rnorm(rnds[p])
            # Issue the prepass for panel p+PIPE after this panel's broadcast:
            # its DVE/PE ops fill scheduling gaps without ever blocking the
            # current panel's work (static per-engine order).
            rhs = rhsp.tile([P, KCH * PANEL], bf16, tag="rhs")
            # One batched multiply builds the whole Y'^T panel:
            #   rhs[:, (h*2+c)*512 + n] = xT[:, c*512 + n] * bc[:, h*512 + n]
            xT = xTs[p]
            in0 = bass.AP(
                xT.tensor,
                xT.offset,
                [list(xT.ap[0]), [0, H], [PANEL, CHD], [1, PANEL]],
            )
            in1 = bass.AP(
                bc.tensor,
                bc.offset,
                [list(bc.ap[0]), [PANEL, H], [0, CHD], [1, PANEL]],
            )
            nc.vector.tensor_tensor(
                rhs[:].rearrange("q (h c n) -> q h c n", h=H, c=CHD),
                in0,
                in1,
                mybir.AluOpType.mult,
            )
            if p + PIPE < NPANELS:
                prepass_x(p + PIPE)

            ot = outp.tile([P, RBLK * PANEL], f32, tag="ot")
            for r in range(RBLK):
                acc = ps_out.tile([P, PANEL], f32, tag="acc")
                for kc in range(KCH):
                    nc.tensor.matmul(
                        acc[:],
                        lhsT[:, kc * NLOC + r * P : kc * NLOC + (r + 1) * P],
                        rhs[:, kc * PANEL : (kc + 1) * PANEL],
                        start=(kc == 0),
                        stop=(kc == KCH - 1),
                    )
                nc.vector.tensor_copy(
                    ot[:, r * PANEL : (r + 1) * PANEL], acc[:]
                )
                # Last panel: ship each row block as soon as it is ready so
                # the kernel tail is one small DMA, not copy-all-then-DMA.
                if p == NPANELS - 1:
                    nc.sync.dma_start(
                        out[
                            r * P : (r + 1) * P,
                            p * PANEL : (p + 1) * PANEL,
                        ],
                        ot[:, r * PANEL : (r + 1) * PANEL],
                    )
            if p != NPANELS - 1:
                nc.sync.dma_start(
                    out[:, p * PANEL : (p + 1) * PANEL].rearrange(
                        "(r q) c -> q r c", q=P
                    ),
                    ot[:].rearrange("q (r c) -> q r c", r=RBLK),
                )

    nc.compile()
    return nc


def _get_compiled():
    if "nc" not in _COMPILED:
        _COMPILED["nc"] = _build_bass()
    return _COMPILED["nc"]


def host_side_inputs(x, attn):
    """Per-core input maps (w_sq / asq are tiny host-precomputed functions
    of attn_vectors; see _build_bass)."""
    import ml_dtypes

    w_sq = np.zeros((P, CHD * H), dtype=np.float32)
    asq = np.zeros((P, KCH), dtype=np.float32)
    for c in range(CHD):
        w_sq[:, c * H : (c + 1) * H] = (attn[:, c * P : (c + 1) * P] ** 2).T
    for kc in range(KCH):
        h, c = divmod(kc, CHD)
        asq[:, kc] = 0.25 * attn[h, c * P : (c + 1) * P] ** 2
    w_sq = w_sq.astype(ml_dtypes.bfloat16)
    return [
        {
            "x": x,
            "x_local": np.ascontiguousarray(x[c * NLOC : (c + 1) * NLOC]),
            "w_sq": w_sq,
            "asq": asq,
        }
        for c in range(NCORES)
    ]


def kernel(**inputs) -> np.ndarray:
    from concourse import bass_utils

    x = np.ascontiguousarray(np.asarray(inputs["x"], dtype=np.float32))
    attn = np.ascontiguousarray(
        np.asarray(inputs["attn_vectors"], dtype=np.float32)
    )
    nc = _get_compiled()
    res = bass_utils.run_bass_kernel_spmd(
        nc, host_side_inputs(x, attn), core_ids=list(range(NCORES))
    )
    out = np.concatenate([r["out"] for r in res.results], axis=0)
    # The exact result is symmetric; the bf16 rounding errors of the two
    # triangles are independent, so symmetrizing averages them down.
    return ((out + out.T) * 0.5).astype(np.float32)



# revision 4
# speedup vs baseline: 1.6153x; 1.6153x over previous
"""Self-contained Trainium2 Bass kernel for the "Attentive" GNN message-passing
problem:

    x: [8192, 256] f32, attn_vectors: [4, 256] f32
    e_h = l2_normalize(attn_vectors[h] * x, axis=-1)        # [H, N, D]
    Y   = concat_h(e_h)                                     # [N, H*D]
    out = (Y @ Y.T) / H                                     # [N, N]

Strategy (8 NeuronCores, SPMD, no collectives):
  - out = Ytil @ diag(a^2) @ Ytil^T / H with Ytil_h = diag(r_h) X (no a
    factor), r_h(n) = 1/sqrt(max(sum_d (a_h[d] x[n,d])^2, eps)).  The a^2
    weight is folded into the lhs side only.
  - The output is symmetric: only the 136 upper-triangle 512x512 blocks of
    the 16x16 block grid are computed, 17 per core.  Core c owns lhs panels
    {c, c+8} and computes blocks (c, c+d) d=0..8 and (c+8, c+8+d) d=0..7
    (mod 16) -- a cyclic-distance covering.  The host rotates the input
    rows by 512*c per core, so the compiled program is identical on all
    cores (block indices become fixed SBUF slots).
  - Matmuls run in fp8e4 (TRN e4m3) DoubleRow mode: 2 K-subtiles of 128
    per instruction at 0.5 cycles/row.  DoubleRow disables fast weight
    load, so the block loops are kp-outer / column-inner: consecutive
    matmuls share the same stationary operand and columns are processed in
    batches of 4 (3 PSUM tiles of 2 banks rotate; 1 bank each for the norm
    pipeline).
  - rhs = e4m3(S * Ytil^T) is built with one batched elementwise multiply
    per panel (xT tile x broadcast rnorm) into a single resident
    [128, 8, 8192] fp8 mega-tile, split DVE/GpSimd by column range
    (fp8-writing ops run at 1x on DVE, so GpSimd takes a share).
    lhs = e4m3(asq * tmp) via ACT per-partition-scaled copies, with
    tmp = bf16(S * Ytil^T) built independently to decorrelate quantization
    noise.
  - Row norms: xsq = xT^2 (DVE), PE matmuls against w_sq = a^2 (bf16) into
    [128, 16] PSUM, clamp/sqrt/reciprocal chain in full-lane layout, PE
    transpose, fp16 DRAM bounce, and one broadcast DMA per panel.
  - The host passes x already transposed and cast to bf16 (pure
    layout/dtype transform; the device would round to bf16 before the PE
    anyway), eliminating all on-device transposes of x.
  - Output blocks leave as fp16 (PSUM->SBUF copy applies 1/(S^2 beta H),
    split ACT/DVE), host scatters blocks + mirrors into the full f32
    matrix.  The true diagonal is exactly 1.0 (rows are L2-normalized), so
    it is overwritten exactly.
"""

from contextlib import ExitStack

import numpy as np

N, D, H = 8192, 256, 4
NCORES = 8
P = 128
PANEL = 512
NPAN = N // PANEL  # 16 row/col panels
CHD = D // P  # 2 c-chunks per head
KCH = H * CHD  # 8 contraction chunks of 128
SUB = PANEL // P  # 4 n-subchunks per panel
NBLK = 17  # upper-tri 512x512 blocks per core
EPS = 1e-12

S = 32.0  # rnorm scale folded into bc (keeps fp8 operands in range)
BETA = 4.0  # extra lhs scale
ALPHA = 1.0 / (S * S * BETA * H)  # PSUM -> out scale
N_POOL = 240  # columns of each rhs panel built on GpSimd (rest on DVE)

# (lhs index, [column-group slots]) per batch; A = slots 0..8, B = 8..15.
BATCHES = [
    (0, [0, 1, 2, 3]),
    (0, [4, 5, 6, 7]),
    (0, [8]),
    (1, [8, 9, 10, 11]),
    (1, [12, 13, 14, 15]),
]
# block t (host order): A d=0..8 -> t=d; B d=0..7 -> t=9+d

_COMPILED = {}


def _build_bass():
    import concourse.bass as bass
    import concourse.tile as tile
    from concourse import bacc, mybir
    from concourse.masks import make_identity

    f32 = mybir.dt.float32
    bf16 = mybir.dt.bfloat16
    fp16 = mybir.dt.float16
    fp8 = mybir.dt.float8e4
    DR = mybir.MatmulPerfMode.DoubleRow
    Copy = mybir.ActivationFunctionType.Copy
    Sqrt = mybir.ActivationFunctionType.Sqrt

    nc = bacc.Bacc(
        "TRN2",
        target_bir_lowering=False,
        debug=False,
        enable_asserts=False,
        num_devices=NCORES,
    )
    # xt[c, d, n] = x_rot[n, c*128+d] (bf16, host-transposed)
    xt_t = nc.dram_tensor("xt", [CHD, P, N], bf16, kind="ExternalInput")
    # w_sq[d, c*H+h] = bf16(attn[h, c*128+d])^2  (norm matmul moving operand)
    ws_t = nc.dram_tensor("w_sq", [P, CHD * H], bf16, kind="ExternalInput")
    # asq[d, h*CHD+c] = BETA * attn[h, c*128+d]^2  (lhs per-partition scale)
    aq_t = nc.dram_tensor("asq", [P, KCH], f32, kind="ExternalInput")
    out_t = nc.dram_tensor("out", [NBLK, PANEL, PANEL], fp16, kind="ExternalOutput")
    xt, out = xt_t.ap(), out_t.ap()

    with tile.TileContext(nc) as tc, ExitStack() as ctx:
        consts = ctx.enter_context(tc.tile_pool(name="consts", bufs=1))
        xsqp = ctx.enter_context(tc.tile_pool(name="xsqp", bufs=2))
        small = ctx.enter_context(tc.tile_pool(name="small", bufs=3))
        bcp = ctx.enter_context(tc.tile_pool(name="bcp", bufs=3))
        tmpp = ctx.enter_context(tc.tile_pool(name="tmpp", bufs=2))
        otp = ctx.enter_context(tc.tile_pool(name="otp", bufs=3))
        dram = ctx.enter_context(tc.tile_pool(name="dram", bufs=1, space="DRAM"))
        ps_pn = ctx.enter_context(tc.tile_pool(name="ps_pn", bufs=1, space="PSUM"))
        ps_tp = ctx.enter_context(tc.tile_pool(name="ps_tp", bufs=1, space="PSUM"))
        ps_out = ctx.enter_context(tc.tile_pool(name="ps_out", bufs=3, space="PSUM"))

        w_sq = consts.tile([P, CHD * H], bf16)
        nc.sync.dma_start(w_sq[:], ws_t.ap()[:])
        asq = consts.tile([P, KCH], f32)
        nc.sync.dma_start(asq[:], aq_t.ap()[:])
        ident = consts.tile([P, P], f32)
        make_identity(nc, ident[:])

        # Full x^T resident (bf16): [d, c, n].  8 n-slab DMAs so panel
        # prepasses can start before the whole load finishes.
        xT = consts.tile([P, CHD, N], bf16, name="xT")
        NSLAB = 1024
        for k in range(N // NSLAB):
            nc.sync.dma_start(
                xT[:, :, k * NSLAB : (k + 1) * NSLAB],
                xt[:, :, k * NSLAB : (k + 1) * NSLAB].rearrange("c q n -> q c n"),
            )

        # One resident fp8 rhs mega-tile: [d, kc=(h,c), n] over all 16 panels.
        rhs = consts.tile([P, KCH, N], fp8, name="rhs")
        lhs_q = [
            consts.tile([P, KCH, PANEL], fp8, name=f"lhs{i}") for i in range(2)
        ]

        def yt_build(eng, out_ap, p, n0, n1, bc):
            """out[(h,c), n] = xT[c, p*512+n0+n] * bc[h, n0+n] (n1-n0 wide)."""
            eng.tensor_tensor(
                out_ap,
                bass.AP(
                    xT.tensor,
                    xT.offset + p * PANEL + n0,
                    [list(xT.ap[0]), [0, H], [N, CHD], [1, n1 - n0]],
                ),
                bass.AP(
                    bc.tensor,
                    bc.offset + n0,
                    [list(bc.ap[0]), [PANEL, H], [0, CHD], [1, n1 - n0]],
                ),
                mybir.AluOpType.mult,
            )

        def prepass(p):
            """Build rhs[:, :, p*512:(p+1)*512] = e4m3(S * Ytil^T panel p);
            returns the bc tile (rnorm broadcast) for lhs reuse."""
            xTp = xT[:, :, p * PANEL : (p + 1) * PANEL]
            xsq = xsqp.tile([P, CHD, PANEL], bf16, tag="xsq")
            nc.vector.tensor_tensor(xsq[:], xTp, xTp, mybir.AluOpType.mult)
            pn = ps_pn.tile([P, SUB * H], f32, tag="pn")
            for i in range(SUB):
                for c in range(CHD):
                    nc.tensor.matmul(
                        pn[:, i * H : (i + 1) * H],
                        xsq[:, c, i * P : (i + 1) * P],
                        w_sq[:, c * H : (c + 1) * H],
                        start=(c == 0),
                        stop=(c == CHD - 1),
                    )
            # clamp also permutes [q,(i h)] -> [q,(h i)] so the transposed
            # store is h-major flat (rnd[h*512 + i*128 + q] = rnorm_h).
            clamped = small.tile([P, SUB * H], f32, tag="clamped")
            nc.vector.tensor_scalar_max(
                clamped[:], pn[:].rearrange("q (i h) -> q h i", h=H), EPS
            )
            root = small.tile([P, SUB * H], f32, tag="root")
            nc.scalar.activation(root[:], clamped[:], Sqrt, scale=1.0 / (S * S))
            rnorm = small.tile([P, SUB * H], f32, tag="rnorm")
            nc.vector.reciprocal(rnorm[:], root[:])
            tp = ps_tp.tile([SUB * H, P], f32, tag="tp")
            nc.tensor.transpose(tp[:], rnorm[:], ident[:])
            rno = small.tile([SUB * H, P], fp16, tag="rno")
            nc.vector.tensor_copy(rno[:], tp[:])
            rnd = dram.tile([SUB * H, P], fp16, name=f"rnd{p}")
            nc.sync.dma_start(rnd[:], rno[:])
            # bc[d, h, n] = S * r_h(panel n)  (stride-0 partition DMA)
            bc = bcp.tile([P, H, PANEL], fp16, tag="bc")
            nc.sync.dma_start(
                bc[:],
                bass.AP(rnd.tensor, rnd.offset, [[0, P], [PANEL, H], [1, PANEL]]),
            )
            out_sl = rhs[:, :, p * PANEL : (p + 1) * PANEL]
            sl = lambda n0, n1: out_sl[:, :, n0:n1].rearrange(
                "q (h c) n -> q h c n", h=H
            )
            yt_build(nc.vector, sl(0, PANEL - N_POOL), p, 0, PANEL - N_POOL, bc)
            yt_build(nc.gpsimd, sl(PANEL - N_POOL, PANEL), p, PANEL - N_POOL, PANEL, bc)
            return bc

        def lhs_build(i, slot, bc):
            """lhs_q[i] = e4m3(asq * bf16(S * Ytil^T panel slot))."""
            tmp = tmpp.tile([P, KCH, PANEL], bf16, tag="tmp")
            yt_build(
                nc.vector,
                tmp[:].rearrange("q (h c) n -> q h c n", h=H),
                slot,
                0,
                PANEL,
                bc,
            )
            for kc in range(KCH):
                nc.scalar.activation(
                    lhs_q[i][:, kc, :], tmp[:, kc, :], Copy, scale=asq[:, kc : kc + 1]
                )

        def batch(li, slots, t0):
            """kp-outer/column-inner block matmuls for one lhs panel x a
            batch of column groups, then ship rows [m] x blocks [t0..)."""
            npt = (len(slots) + 1) // 2
            for m in range(SUB):
                pts = [
                ps_out.tile([P, 2, PANEL], f32, tag="acc", name=f"acc{li}_{m}_{g}")
                for g in range(npt)
            ]
                for kp in range(KCH // 2):
                    for g, s in enumerate(slots):
                        nc.tensor.matmul(
                            pts[g // 2][:, g % 2, :],
                            lhs_q[li][:, 2 * kp : 2 * kp + 2, m * P : (m + 1) * P],
                            rhs[:, 2 * kp : 2 * kp + 2, s * PANEL : (s + 1) * PANEL],
                            start=(kp == 0),
                            stop=(kp == KCH // 2 - 1),
                            perf_mode=DR,
                        )
                ot = otp.tile([P, len(slots) * PANEL], fp16, tag="ot")
                for g in range(npt):
                    ncols = min(2, len(slots) - 2 * g) * PANEL
                    dst = ot[:, 2 * g * PANEL : 2 * g * PANEL + ncols]
                    src = pts[g][:].rearrange("q two n -> q (two n)")[:, :ncols]
                    if g % 2:
                        nc.vector.tensor_scalar_mul(dst, src, ALPHA)
                    else:
                        nc.scalar.activation(dst, src, Copy, scale=ALPHA)
                nc.sync.dma_start(
                    out[t0 : t0 + len(slots), m * P : (m + 1) * P, :].rearrange(
                        "t q n -> q t n"
                    ),
                    ot[:].rearrange("q (t n) -> q t n", n=PANEL),
                )

        # ---- emission: prepass pipeline feeding the block pipeline -------
        bcA = prepass(0)
        bcB = prepass(8)
        lhs_build(0, 0, bcA)
        lhs_build(1, 8, bcB)
        for p in (1, 2, 3, 9):
            prepass(p)
        batch(0, [0, 1, 2, 3], 0)
        for p in (4, 5, 6, 7):
            prepass(p)
        batch(0, [4, 5, 6, 7], 4)
        for p in (10, 11, 12, 13):
            prepass(p)
        batch(0, [8], 8)
        batch(1, [8, 9, 10, 11], 9)
        for p in (14, 15):
            prepass(p)
        batch(1, [12, 13, 14, 15], 13)

    nc.compile()
    return nc


def _get_compiled():
    if "nc" not in _COMPILED:
        _COMPILED["nc"] = _build_bass()
    return _COMPILED["nc"]


def host_side_inputs(x, attn):
    """Per-core input maps. w_sq/asq are tiny host-precomputed functions of
    attn_vectors; xt is a per-core rotated, transposed bf16 copy of x."""
    import ml_dtypes

    bf16 = ml_dtypes.bfloat16
    ab = attn.astype(bf16).astype(np.float32)
    w_sq = np.zeros((P, CHD * H), dtype=np.float32)
    for c in range(CHD):
        w_sq[:, c * H : (c + 1) * H] = (ab[:, c * P : (c + 1) * P] ** 2).T
    w_sq = w_sq.astype(bf16)
    asq = np.zeros((P, KCH), dtype=np.float32)
    for h in range(H):
        for c in range(CHD):
            asq[:, h * CHD + c] = BETA * attn[h, c * P : (c + 1) * P] ** 2
    xb = x.astype(bf16)
    ins = []
    for cid in range(NCORES):
        xr = np.roll(xb, -PANEL * cid, axis=0)  # [N, D] bf16
        xtc = np.ascontiguousarray(xr.T).reshape(CHD, P, N)
        ins.append({"xt": xtc, "w_sq": w_sq, "asq": asq})
    return ins


def _core_blocks(cid):
    """[(t, pi, pj)] global block positions for the 17 blocks of core cid."""
    blocks = []
    for t in range(NBLK):
        if t <= 8:
            pi, pj = cid, (cid + t) % NPAN
        else:
            pi, pj = cid + 8, (cid + 8 + (t - 9)) % NPAN
        blocks.append((t, pi, pj))
    return blocks


def assemble(results):
    out = np.empty((N, N), dtype=np.float32)
    for cid in range(NCORES):
        blks = np.asarray(results[cid]["out"]).astype(np.float32)
        for t, pi, pj in _core_blocks(cid):
            b = blks[t]
            ri = slice(pi * PANEL, (pi + 1) * PANEL)
            rj = slice(pj * PANEL, (pj + 1) * PANEL)
            if pi == pj:
                out[ri, rj] = 0.5 * (b + b.T)
            else:
                out[ri, rj] = b
                out[rj, ri] = b.T
    # rows are L2-normalized: diag(Y Y^T / H) == 1 exactly
    np.fill_diagonal(out, 1.0)
    return out


def kernel(**inputs) -> np.ndarray:
    from concourse import bass_utils

    x = np.ascontiguousarray(np.asarray(inputs["x"], dtype=np.float32))
    attn = np.ascontiguousarray(np.asarray(inputs["attn_vectors"], dtype=np.float32))
    nc = _get_compiled()
    res = bass_utils.run_bass_kernel_spmd(
        nc, host_side_inputs(x, attn), core_ids=list(range(NCORES))
    )
    return assemble(res.results)


# revision 8
# speedup vs baseline: 2.0018x; 1.2392x over previous
"""Self-contained Trainium2 Bass kernel for the "Attentive" GNN message-passing
problem:

    x: [8192, 256] f32, attn_vectors: [4, 256] f32
    e_h = l2_normalize(attn_vectors[h] * x, axis=-1)        # [H, N, D]
    Y   = concat_h(e_h)                                     # [N, H*D]
    out = (Y @ Y.T) / H                                     # [N, N]

Strategy (8 NeuronCores, SPMD, no collectives):
  - out = Ytil @ diag(a^2) @ Ytil^T / H with Ytil_h = diag(r_h) X (no a
    factor), r_h(n) = 1/sqrt(max(sum_d (a_h[d] x[n,d])^2, eps)).  The a^2
    weight is folded into the lhs side only.
  - The output is symmetric: only the 136 upper-triangle 512x512 blocks of
    the 16x16 block grid are computed, 17 per core.  Core c owns lhs panels
    {c, c+8} and computes blocks (c, c+d) d=0..8 and (c+8, c+8+d) d=0..7
    (mod 16) -- a cyclic-distance covering.  The host rotates the input
    rows by 512*c per core, so the compiled program is identical on all
    cores (block indices become fixed SBUF slots).
  - Matmuls run in fp8e4 (TRN e4m3) DoubleRow mode: 2 K-subtiles of 128
    per instruction at 0.5 cycles/row.  DoubleRow disables fast weight
    load, so the block loops are kp-outer / column-inner: consecutive
    matmuls share the same stationary operand and columns are processed in
    batches of 4 (3 PSUM tiles of 2 banks rotate; 1 bank each for the norm
    pipeline).
  - rhs = e4m3(S * Ytil^T) is built with one batched elementwise multiply
    per panel (xT tile x broadcast rnorm) into a single resident
    [128, 8, 8192] fp8 mega-tile, split DVE/GpSimd by column range
    (fp8-writing ops run at 1x on DVE, so GpSimd takes a share).
    lhs = e4m3(asq * tmp) via ACT per-partition-scaled copies, with
    tmp = bf16(S * Ytil^T) built independently to decorrelate quantization
    noise.
  - Row norms: xsq = xT^2 (DVE), PE matmuls against w_sq = a^2 (bf16) into
    [128, 16] PSUM, clamp/sqrt/reciprocal chain in full-lane layout, PE
    transpose, fp16 DRAM bounce, and one broadcast DMA per panel.
  - The host passes x already transposed and cast to bf16 (pure
    layout/dtype transform; the device would round to bf16 before the PE
    anyway), eliminating all on-device transposes of x.
  - Output blocks leave as fp16 (PSUM->SBUF copy applies 1/(S^2 beta H),
    split ACT/DVE), host scatters blocks + mirrors into the full f32
    matrix.  The true diagonal is exactly 1.0 (rows are L2-normalized), so
    it is overwritten exactly.
"""

from contextlib import ExitStack

import numpy as np

N, D, H = 8192, 256, 4
NCORES = 8
P = 128
PANEL = 512
NPAN = N // PANEL  # 16 row/col panels
CHD = D // P  # 2 c-chunks per head
KCH = H * CHD  # 8 contraction chunks of 128
SUB = PANEL // P  # 4 n-subchunks per panel
NBLK = 17  # upper-tri 512x512 blocks per core
EPS = 1e-12

S = 32.0  # rnorm scale folded into bc (keeps fp8 operands in range)
BETA = 4.0  # extra lhs scale
ALPHA = 1.0 / (S * S * BETA * H)  # PSUM -> out scale
# GpSimd is kept OFF the hot path: its SBUF ports are shared with DVE, and
# measured Pool multiplies (~2.9 ns/elem) stall concurrent DVE ops to Pool's
# speed.  The fp8-writing DVE builds measure ~0.27 ns/elem, so DVE alone wins.
N_POOL = 0  # columns of each rhs panel built on GpSimd (rest on DVE)

# (lhs index, [column-group slots]) per batch; A = slots 0..8, B = 8..15.
BATCHES = [
    (0, [0, 1, 2, 3]),
    (0, [4, 5, 6, 7]),
    (0, [8]),
    (1, [8, 9, 10, 11]),
    (1, [12, 13, 14, 15]),
]
# block t (host order): A d=0..8 -> t=d; B d=0..7 -> t=9+d

_COMPILED = {}


def _build_bass():
    import concourse.bass as bass
    import concourse.tile as tile
    from concourse import bacc, mybir
    from concourse.masks import make_identity

    f32 = mybir.dt.float32
    bf16 = mybir.dt.bfloat16
    fp16 = mybir.dt.float16
    fp8 = mybir.dt.float8e4
    DR = mybir.MatmulPerfMode.DoubleRow
    Copy = mybir.ActivationFunctionType.Copy
    Sqrt = mybir.ActivationFunctionType.Sqrt

    nc = bacc.Bacc(
        "TRN2",
        target_bir_lowering=False,
        debug=False,
        enable_asserts=False,
        num_devices=NCORES,
    )
    # xt[c, d, n] = x_rot[n, c*128+d] (bf16, host-transposed)
    xt_t = nc.dram_tensor("xt", [CHD, P, N], bf16, kind="ExternalInput")
    # w_sq[d, c*H+h] = bf16(attn[h, c*128+d])^2  (norm matmul moving operand)
    ws_t = nc.dram_tensor("w_sq", [P, CHD * H], bf16, kind="ExternalInput")
    # asq[d, h*CHD+c] = BETA * attn[h, c*128+d]^2  (lhs per-partition scale)
    aq_t = nc.dram_tensor("asq", [P, KCH], f32, kind="ExternalInput")
    out_t = nc.dram_tensor("out", [NBLK, PANEL, PANEL], fp16, kind="ExternalOutput")
    xt, out = xt_t.ap(), out_t.ap()

    with tile.TileContext(nc) as tc, ExitStack() as ctx:
        consts = ctx.enter_context(tc.tile_pool(name="consts", bufs=1))
        xsqp = ctx.enter_context(tc.tile_pool(name="xsqp", bufs=2))
        small = ctx.enter_context(tc.tile_pool(name="small", bufs=3))
        bcp = ctx.enter_context(tc.tile_pool(name="bcp", bufs=3))
        tmpp = ctx.enter_context(tc.tile_pool(name="tmpp", bufs=2))
        otp = ctx.enter_context(tc.tile_pool(name="otp", bufs=3))
        dram = ctx.enter_context(tc.tile_pool(name="dram", bufs=1, space="DRAM"))
        ps_pn = ctx.enter_context(tc.tile_pool(name="ps_pn", bufs=1, space="PSUM"))
        ps_tp = ctx.enter_context(tc.tile_pool(name="ps_tp", bufs=1, space="PSUM"))
        ps_out = ctx.enter_context(tc.tile_pool(name="ps_out", bufs=3, space="PSUM"))

        w_sq = consts.tile([P, CHD * H], bf16)
        nc.sync.dma_start(w_sq[:], ws_t.ap()[:])
        asq = consts.tile([P, KCH], f32)
        nc.sync.dma_start(asq[:], aq_t.ap()[:])
        ident = consts.tile([P, P], f32)
        make_identity(nc, ident[:])

        # Full x^T resident (bf16): [d, c, n].  8 n-slab DMAs so panel
        # prepasses can start before the whole load finishes.
        xT = consts.tile([P, CHD, N], bf16, name="xT")
        NSLAB = 1024
        for k in range(N // NSLAB):
            nc.sync.dma_start(
                xT[:, :, k * NSLAB : (k + 1) * NSLAB],
                xt[:, :, k * NSLAB : (k + 1) * NSLAB].rearrange("c q n -> q c n"),
            )

        # One resident fp8 rhs mega-tile: [d, kc=(h,c), n] over all 16 panels.
        rhs = consts.tile([P, KCH, N], fp8, name="rhs")
        lhs_q = [
            consts.tile([P, KCH, PANEL], fp8, name=f"lhs{i}") for i in range(2)
        ]

        def yt_build(eng, out_ap, p, n0, n1, bc):
            """out[(h,c), n] = xT[c, p*512+n0+n] * bc[h, n0+n] (n1-n0 wide)."""
            eng.tensor_tensor(
                out_ap,
                bass.AP(
                    xT.tensor,
                    xT.offset + p * PANEL + n0,
                    [list(xT.ap[0]), [0, H], [N, CHD], [1, n1 - n0]],
                ),
                bass.AP(
                    bc.tensor,
                    bc.offset + n0,
                    [list(bc.ap[0]), [PANEL, H], [0, CHD], [1, n1 - n0]],
                ),
                mybir.AluOpType.mult,
            )

        def prepass(p):
            """Build rhs[:, :, p*512:(p+1)*512] = e4m3(S * Ytil^T panel p);
            returns the bc tile (rnorm broadcast) for lhs reuse."""
            xTp = xT[:, :, p * PANEL : (p + 1) * PANEL]
            xsq = xsqp.tile([P, CHD, PANEL], bf16, tag="xsq")
            nc.vector.tensor_tensor(xsq[:], xTp, xTp, mybir.AluOpType.mult)
            pn = ps_pn.tile([P, SUB * H], f32, tag="pn")
            for i in range(SUB):
                for c in range(CHD):
                    nc.tensor.matmul(
                        pn[:, i * H : (i + 1) * H],
                        xsq[:, c, i * P : (i + 1) * P],
                        w_sq[:, c * H : (c + 1) * H],
                        start=(c == 0),
                        stop=(c == CHD - 1),
                    )
            # clamp also permutes [q,(i h)] -> [q,(h i)] so the transposed
            # store is h-major flat (rnd[h*512 + i*128 + q] = rnorm_h).
            clamped = small.tile([P, SUB * H], f32, tag="clamped")
            nc.vector.tensor_scalar_max(
                clamped[:], pn[:].rearrange("q (i h) -> q h i", h=H), EPS
            )
            root = small.tile([P, SUB * H], f32, tag="root")
            nc.scalar.activation(root[:], clamped[:], Sqrt, scale=1.0 / (S * S))
            rnorm = small.tile([P, SUB * H], f32, tag="rnorm")
            nc.vector.reciprocal(rnorm[:], root[:])
            tp = ps_tp.tile([SUB * H, P], f32, tag="tp")
            nc.tensor.transpose(tp[:], rnorm[:], ident[:])
            rno = small.tile([SUB * H, P], fp16, tag="rno")
            nc.vector.tensor_copy(rno[:], tp[:])
            rnd = dram.tile([SUB * H, P], fp16, name=f"rnd{p}")
            nc.sync.dma_start(rnd[:], rno[:])
            # bc[d, h, n] = S * r_h(panel n)  (stride-0 partition DMA)
            bc = bcp.tile([P, H, PANEL], fp16, tag="bc")
            nc.sync.dma_start(
                bc[:],
                bass.AP(rnd.tensor, rnd.offset, [[0, P], [PANEL, H], [1, PANEL]]),
            )
            out_sl = rhs[:, :, p * PANEL : (p + 1) * PANEL]
            sl = lambda n0, n1: out_sl[:, :, n0:n1].rearrange(
                "q (h c) n -> q h c n", h=H
            )
            if N_POOL:
                yt_build(nc.vector, sl(0, PANEL - N_POOL), p, 0, PANEL - N_POOL, bc)
                yt_build(
                    nc.gpsimd, sl(PANEL - N_POOL, PANEL), p, PANEL - N_POOL, PANEL, bc
                )
            else:
                yt_build(nc.vector, sl(0, PANEL), p, 0, PANEL, bc)
            return bc

        def lhs_build(i, slot, bc):
            """lhs_q[i] = e4m3(asq * bf16(S * Ytil^T panel slot))."""
            tmp = tmpp.tile([P, KCH, PANEL], bf16, tag="tmp")
            yt_build(
                nc.vector,
                tmp[:].rearrange("q (h c) n -> q h c n", h=H),
                slot,
                0,
                PANEL,
                bc,
            )
            for kc in range(KCH):
                nc.scalar.activation(
                    lhs_q[i][:, kc, :], tmp[:, kc, :], Copy, scale=asq[:, kc : kc + 1]
                )

        def batch(li, slots, t0):
            """kp-outer/column-inner block matmuls for one lhs panel x a
            batch of column groups, then ship rows [m] x blocks [t0..)."""
            npt = (len(slots) + 1) // 2
            for m in range(SUB):
                pts = [
                ps_out.tile([P, 2, PANEL], f32, tag="acc", name=f"acc{li}_{m}_{g}")
                for g in range(npt)
            ]
                for kp in range(KCH // 2):
                    for g, s in enumerate(slots):
                        nc.tensor.matmul(
                            pts[g // 2][:, g % 2, :],
                            lhs_q[li][:, 2 * kp : 2 * kp + 2, m * P : (m + 1) * P],
                            rhs[:, 2 * kp : 2 * kp + 2, s * PANEL : (s + 1) * PANEL],
                            start=(kp == 0),
                            stop=(kp == KCH // 2 - 1),
                            perf_mode=DR,
                        )
                ot = otp.tile([P, len(slots) * PANEL], fp16, tag="ot")
                for g in range(npt):
                    ncols = min(2, len(slots) - 2 * g) * PANEL
                    dst = ot[:, 2 * g * PANEL : 2 * g * PANEL + ncols]
                    src = pts[g][:].rearrange("q two n -> q (two n)")[:, :ncols]
                    # PSUM reads from DVE measured 5.8 ns/elem under port
                    # contention; ACT does the same copy in ~1 us.
                    nc.scalar.activation(dst, src, Copy, scale=ALPHA)
                nc.sync.dma_start(
                    out[t0 : t0 + len(slots), m * P : (m + 1) * P, :].rearrange(
                        "t q n -> q t n"
                    ),
                    ot[:].rearrange("q (t n) -> q t n", n=PANEL),
                )

        # ---- emission: prepass pipeline feeding the block pipeline -------
        bcA = prepass(0)
        bcB = prepass(8)
        with tc.high_priority():
            lhs_build(0, 0, bcA)
            lhs_build(1, 8, bcB)
        for p in (1, 2, 3, 9):
            prepass(p)
        batch(0, [0, 1, 2, 3], 0)
        for p in (4, 5, 6, 7):
            prepass(p)
        batch(0, [4, 5, 6, 7], 4)
        for p in (10, 11, 12, 13):
            prepass(p)
        batch(0, [8], 8)
        batch(1, [8, 9, 10, 11], 9)
        for p in (14, 15):
            prepass(p)
        batch(1, [12, 13, 14, 15], 13)

    nc.compile()
    return nc


def _get_compiled():
    if "nc" not in _COMPILED:
        _COMPILED["nc"] = _build_bass()
    return _COMPILED["nc"]


def host_side_inputs(x, attn):
    """Per-core input maps. w_sq/asq are tiny host-precomputed functions of
    attn_vectors; xt is a per-core rotated, transposed bf16 copy of x."""
    import ml_dtypes

    bf16 = ml_dtypes.bfloat16
    ab = attn.astype(bf16).astype(np.float32)
    w_sq = np.zeros((P, CHD * H), dtype=np.float32)
    for c in range(CHD):
        w_sq[:, c * H : (c + 1) * H] = (ab[:, c * P : (c + 1) * P] ** 2).T
    w_sq = w_sq.astype(bf16)
    asq = np.zeros((P, KCH), dtype=np.float32)
    for h in range(H):
        for c in range(CHD):
            asq[:, h * CHD + c] = BETA * attn[h, c * P : (c + 1) * P] ** 2
    xb = x.astype(bf16)
    ins = []
    for cid in range(NCORES):
        xr = np.roll(xb, -PANEL * cid, axis=0)  # [N, D] bf16
        xtc = np.ascontiguousarray(xr.T).reshape(CHD, P, N)
        ins.append({"xt": xtc, "w_sq": w_sq, "asq": asq})
    return ins


def _core_blocks(cid):
    """[(t, pi, pj)] global block positions for the 17 blocks of core cid."""
    blocks = []
    for t in range(NBLK):
        if t <= 8:
            pi, pj = cid, (cid + t) % NPAN
        else:
            pi, pj = cid + 8, (cid + 8 + (t - 9)) % NPAN
        blocks.append((t, pi, pj))
    return blocks


def assemble(results):
    out = np.empty((N, N), dtype=np.float32)
    for cid in range(NCORES):
        blks = np.asarray(results[cid]["out"]).astype(np.float32)
        for t, pi, pj in _core_blocks(cid):
            b = blks[t]
            ri = slice(pi * PANEL, (pi + 1) * PANEL)
            rj = slice(pj * PANEL, (pj + 1) * PANEL)
            if pi == pj:
                out[ri, rj] = 0.5 * (b + b.T)
            else:
                out[ri, rj] = b
                out[rj, ri] = b.T
    # rows are L2-normalized: diag(Y Y^T / H) == 1 exactly
    np.fill_diagonal(out, 1.0)
    return out


def kernel(**inputs) -> np.ndarray:
    from concourse import bass_utils

    x = np.ascontiguousarray(np.asarray(inputs["x"], dtype=np.float32))
    attn = np.ascontiguousarray(np.asarray(inputs["attn_vectors"], dtype=np.float32))
    nc = _get_compiled()
    res = bass_utils.run_bass_kernel_spmd(
        nc, host_side_inputs(x, attn), core_ids=list(range(NCORES))
    )
    return assemble(res.results)


# revision 12
# speedup vs baseline: 2.1641x; 1.0811x over previous
"""Self-contained Trainium2 Bass kernel for the "Attentive" GNN message-passing
problem:

    x: [8192, 256] f32, attn_vectors: [4, 256] f32
    e_h = l2_normalize(attn_vectors[h] * x, axis=-1)        # [H, N, D]
    Y   = concat_h(e_h)                                     # [N, H*D]
    out = (Y @ Y.T) / H                                     # [N, N]

Strategy (8 NeuronCores, SPMD, no collectives):
  - out = Ytil @ diag(a^2) @ Ytil^T / H with Ytil_h = diag(r_h) X (no a
    factor), r_h(n) = 1/sqrt(max(sum_d (a_h[d] x[n,d])^2, eps)).  The a^2
    weight is folded into the lhs side only.
  - The output is symmetric: only the 136 upper-triangle 512x512 blocks of
    the 16x16 block grid are computed, 17 per core.  Core c owns lhs panels
    {c, c+8} and computes blocks (c, c+d) d=0..8 and (c+8, c+8+d) d=0..7
    (mod 16) -- a cyclic-distance covering.  The host rotates the input
    rows by 512*c per core, so the compiled program is identical on all
    cores (block indices become fixed SBUF slots).
  - Matmuls run in fp8e4 (TRN e4m3) DoubleRow mode: 2 K-subtiles of 128
    per instruction at 0.5 cycles/row.  DoubleRow disables fast weight
    load, so the block loops are kp-outer / column-inner: consecutive
    matmuls share the same stationary operand and columns are processed in
    batches of 4 (3 PSUM tiles of 2 banks rotate; 1 bank each for the norm
    pipeline).
  - rhs = e4m3(S * Ytil^T) is built with one batched elementwise multiply
    per panel (xT tile x broadcast rnorm) into a single resident
    [128, 8, 8192] fp8 mega-tile, split DVE/GpSimd by column range
    (fp8-writing ops run at 1x on DVE, so GpSimd takes a share).
    lhs = e4m3(asq * tmp) via ACT per-partition-scaled copies, with
    tmp = bf16(S * Ytil^T) built independently to decorrelate quantization
    noise.
  - Row norms: xsq = xT^2 (DVE), PE matmuls against w_sq = a^2 (bf16) into
    [128, 16] PSUM, clamp/sqrt/reciprocal chain in full-lane layout, PE
    transpose, fp16 DRAM bounce, and one broadcast DMA per panel.
  - The host passes x already transposed and cast to bf16 (pure
    layout/dtype transform; the device would round to bf16 before the PE
    anyway), eliminating all on-device transposes of x.
  - Output blocks leave as fp16 (PSUM->SBUF copy applies 1/(S^2 beta H),
    split ACT/DVE), host scatters blocks + mirrors into the full f32
    matrix.  The true diagonal is exactly 1.0 (rows are L2-normalized), so
    it is overwritten exactly.
"""

from contextlib import ExitStack

import numpy as np

N, D, H = 8192, 256, 4
NCORES = 8
P = 128
PANEL = 512
NPAN = N // PANEL  # 16 row/col panels
CHD = D // P  # 2 c-chunks per head
KCH = H * CHD  # 8 contraction chunks of 128
SUB = PANEL // P  # 4 n-subchunks per panel
NBLK = 17  # upper-tri 512x512 blocks per core
EPS = 1e-12

S = 32.0  # rnorm scale folded into bc (keeps fp8 operands in range)
BETA = 4.0  # extra lhs scale
ALPHA = 1.0 / (S * S * BETA * H)  # PSUM -> out scale
# GpSimd is kept OFF the hot path: its SBUF ports are shared with DVE, and
# measured Pool multiplies (~2.9 ns/elem) stall concurrent DVE ops to Pool's
# speed.
N_POOL = 0  # columns of each rhs panel built on GpSimd (rest on DVE)
# fp8-writing DVE ops run at 1x (~1.06 ns/elem) vs ~0.56 for bf16 writes, so
# ACT takes part of the fp8 production: these panels build a bf16 tmp on DVE
# and cast to fp8 on ACT.  Panels 0/8 reuse the lhs tmp (their cast is free).
ACT_PANELS = (4, 6, 12, 14)

# (lhs index, [column-group slots]) per batch; A = slots 0..8, B = 8..15.
BATCHES = [
    (0, [0, 1, 2, 3]),
    (0, [4, 5, 6, 7]),
    (0, [8]),
    (1, [8, 9, 10, 11]),
    (1, [12, 13, 14, 15]),
]
# block t (host order): A d=0..8 -> t=d; B d=0..7 -> t=9+d

_COMPILED = {}


def _build_bass():
    import concourse.bass as bass
    import concourse.tile as tile
    from concourse import bacc, mybir
    from concourse.masks import make_identity

    f32 = mybir.dt.float32
    bf16 = mybir.dt.bfloat16
    fp16 = mybir.dt.float16
    fp8 = mybir.dt.float8e4
    DR = mybir.MatmulPerfMode.DoubleRow
    Copy = mybir.ActivationFunctionType.Copy
    Sqrt = mybir.ActivationFunctionType.Sqrt

    nc = bacc.Bacc(
        "TRN2",
        target_bir_lowering=False,
        debug=False,
        enable_asserts=False,
        num_devices=NCORES,
    )
    # xt[c, d, n] = x_rot[n, c*128+d] (bf16, host-transposed)
    xt_t = nc.dram_tensor("xt", [CHD, P, N], bf16, kind="ExternalInput")
    # w_sq[d, c*H+h] = bf16(attn[h, c*128+d])^2  (norm matmul moving operand)
    ws_t = nc.dram_tensor("w_sq", [P, CHD * H], bf16, kind="ExternalInput")
    # asq[d, h*CHD+c] = BETA * attn[h, c*128+d]^2  (lhs per-partition scale)
    aq_t = nc.dram_tensor("asq", [P, KCH], f32, kind="ExternalInput")
    out_t = nc.dram_tensor("out", [NBLK, PANEL, PANEL], fp16, kind="ExternalOutput")
    xt, out = xt_t.ap(), out_t.ap()

    with tile.TileContext(nc) as tc, ExitStack() as ctx:
        consts = ctx.enter_context(tc.tile_pool(name="consts", bufs=1))
        xsqp = ctx.enter_context(tc.tile_pool(name="xsqp", bufs=2))
        small = ctx.enter_context(tc.tile_pool(name="small", bufs=3))
        bcp = ctx.enter_context(tc.tile_pool(name="bcp", bufs=3))
        tmpp = ctx.enter_context(tc.tile_pool(name="tmpp", bufs=2))
        otp = ctx.enter_context(tc.tile_pool(name="otp", bufs=3))
        dram = ctx.enter_context(tc.tile_pool(name="dram", bufs=1, space="DRAM"))
        ps_pn = ctx.enter_context(tc.tile_pool(name="ps_pn", bufs=1, space="PSUM"))
        ps_tp = ctx.enter_context(tc.tile_pool(name="ps_tp", bufs=1, space="PSUM"))
        ps_out = ctx.enter_context(tc.tile_pool(name="ps_out", bufs=3, space="PSUM"))

        w_sq = consts.tile([P, CHD * H], bf16)
        nc.sync.dma_start(w_sq[:], ws_t.ap()[:])
        asq = consts.tile([P, KCH], f32)
        nc.sync.dma_start(asq[:], aq_t.ap()[:])
        ident = consts.tile([P, P], f32)
        make_identity(nc, ident[:])

        # Full x^T resident (bf16): [d, c, n].  8 n-slab DMAs so panel
        # prepasses can start before the whole load finishes.
        xT = consts.tile([P, CHD, N], bf16, name="xT")
        NSLAB = 1024
        for k in range(N // NSLAB):
            nc.sync.dma_start(
                xT[:, :, k * NSLAB : (k + 1) * NSLAB],
                xt[:, :, k * NSLAB : (k + 1) * NSLAB].rearrange("c q n -> q c n"),
            )

        # One resident fp8 rhs mega-tile: [d, kc=(h,c), n] over all 16 panels.
        rhs = consts.tile([P, KCH, N], fp8, name="rhs")
        lhs_q = [
            consts.tile([P, KCH, PANEL], fp8, name=f"lhs{i}") for i in range(2)
        ]

        def yt_build(eng, out_ap, p, n0, n1, bc):
            """out[(h,c), n] = xT[c, p*512+n0+n] * bc[h, n0+n] (n1-n0 wide)."""
            eng.tensor_tensor(
                out_ap,
                bass.AP(
                    xT.tensor,
                    xT.offset + p * PANEL + n0,
                    [list(xT.ap[0]), [0, H], [N, CHD], [1, n1 - n0]],
                ),
                bass.AP(
                    bc.tensor,
                    bc.offset + n0,
                    [list(bc.ap[0]), [PANEL, H], [0, CHD], [1, n1 - n0]],
                ),
                mybir.AluOpType.mult,
            )

        def prepass(p):
            """Build rhs[:, :, p*512:(p+1)*512] = e4m3(S * Ytil^T panel p);
            returns the bc tile (rnorm broadcast) for lhs reuse."""
            xTp = xT[:, :, p * PANEL : (p + 1) * PANEL]
            xsq = xsqp.tile([P, CHD, PANEL], bf16, tag="xsq")
            nc.vector.tensor_tensor(xsq[:], xTp, xTp, mybir.AluOpType.mult)
            pn = ps_pn.tile([P, SUB * H], f32, tag="pn")
            for i in range(SUB):
                for c in range(CHD):
                    nc.tensor.matmul(
                        pn[:, i * H : (i + 1) * H],
                        xsq[:, c, i * P : (i + 1) * P],
                        w_sq[:, c * H : (c + 1) * H],
                        start=(c == 0),
                        stop=(c == CHD - 1),
                    )
            # clamp also permutes [q,(i h)] -> [q,(h i)] so the transposed
            # store is h-major flat (rnd[h*512 + i*128 + q] = rnorm_h).
            clamped = small.tile([P, SUB * H], f32, tag="clamped")
            nc.vector.tensor_scalar_max(
                clamped[:], pn[:].rearrange("q (i h) -> q h i", h=H), EPS
            )
            root = small.tile([P, SUB * H], f32, tag="root")
            nc.scalar.activation(root[:], clamped[:], Sqrt, scale=1.0 / (S * S))
            rnorm = small.tile([P, SUB * H], f32, tag="rnorm")
            nc.vector.reciprocal(rnorm[:], root[:])
            tp = ps_tp.tile([SUB * H, P], f32, tag="tp")
            nc.tensor.transpose(tp[:], rnorm[:], ident[:])
            rno = small.tile([SUB * H, P], fp16, tag="rno")
            nc.vector.tensor_copy(rno[:], tp[:])
            rnd = dram.tile([SUB * H, P], fp16, name=f"rnd{p}")
            nc.sync.dma_start(rnd[:], rno[:])
            # bc[d, h, n] = S * r_h(panel n)  (stride-0 partition DMA)
            bc = bcp.tile([P, H, PANEL], fp16, tag="bc")
            nc.sync.dma_start(
                bc[:],
                bass.AP(rnd.tensor, rnd.offset, [[0, P], [PANEL, H], [1, PANEL]]),
            )
            out_sl = rhs[:, :, p * PANEL : (p + 1) * PANEL]
            sl = lambda n0, n1: out_sl[:, :, n0:n1].rearrange(
                "q (h c) n -> q h c n", h=H
            )
            if p in (0, 8):
                pass  # rhs cast comes from the lhs tmp in lhs_build
            elif p in ACT_PANELS:
                rtmp = tmpp.tile([P, KCH, PANEL], bf16, tag="rtmp")
                yt_build(
                    nc.vector,
                    rtmp[:].rearrange("q (h c) n -> q h c n", h=H),
                    p,
                    0,
                    PANEL,
                    bc,
                )
                nc.scalar.activation(out_sl, rtmp[:], Copy)
            else:
                yt_build(nc.vector, sl(0, PANEL), p, 0, PANEL, bc)
            return bc

        def lhs_build(i, slot, bc):
            """lhs_q[i] = e4m3(asq * bf16(S * Ytil^T panel slot)); the same
            tmp also provides rhs slot `slot` via a plain fp8 cast."""
            tmp = tmpp.tile([P, KCH, PANEL], bf16, tag="tmp")
            yt_build(
                nc.vector,
                tmp[:].rearrange("q (h c) n -> q h c n", h=H),
                slot,
                0,
                PANEL,
                bc,
            )
            nc.scalar.activation(
                rhs[:, :, slot * PANEL : (slot + 1) * PANEL], tmp[:], Copy
            )
            for kc in range(KCH):
                nc.scalar.activation(
                    lhs_q[i][:, kc, :], tmp[:, kc, :], Copy, scale=asq[:, kc : kc + 1]
                )

        def batch(li, slots, t0):
            """kp-outer/column-inner block matmuls for one lhs panel x a
            batch of column groups, then ship rows [m] x blocks [t0..)."""
            npt = (len(slots) + 1) // 2
            for m in range(SUB):
                pts = [
                ps_out.tile([P, 2, PANEL], f32, tag="acc", name=f"acc{li}_{m}_{g}")
                for g in range(npt)
            ]
                for kp in range(KCH // 2):
                    for g, s in enumerate(slots):
                        nc.tensor.matmul(
                            pts[g // 2][:, g % 2, :],
                            lhs_q[li][:, 2 * kp : 2 * kp + 2, m * P : (m + 1) * P],
                            rhs[:, 2 * kp : 2 * kp + 2, s * PANEL : (s + 1) * PANEL],
                            start=(kp == 0),
                            stop=(kp == KCH // 2 - 1),
                            perf_mode=DR,
                        )
                ot = otp.tile([P, len(slots) * PANEL], fp16, tag="ot")
                for g in range(npt):
                    ncols = min(2, len(slots) - 2 * g) * PANEL
                    dst = ot[:, 2 * g * PANEL : 2 * g * PANEL + ncols]
                    src = pts[g][:].rearrange("q two n -> q (two n)")[:, :ncols]
                    # PSUM reads from DVE measured 5.8 ns/elem under port
                    # contention; ACT does the same copy in ~1 us.
                    nc.scalar.activation(dst, src, Copy, scale=ALPHA)
                nc.sync.dma_start(
                    out[t0 : t0 + len(slots), m * P : (m + 1) * P, :].rearrange(
                        "t q n -> q t n"
                    ),
                    ot[:].rearrange("q (t n) -> q t n", n=PANEL),
                )

        # ---- emission: prepass pipeline feeding the block pipeline -------
        # Prepasses get a large priority boost so the scheduler slots their
        # PE/DVE work ahead of queued block matmuls (otherwise later panels'
        # norm matmuls sit behind ~17us of block matmuls and starve DVE).
        def pre(p):
            with tc.high_priority(offset=600):
                return prepass(p)

        bcA = pre(0)
        bcB = pre(8)
        with tc.high_priority():
            lhs_build(0, 0, bcA)
            lhs_build(1, 8, bcB)
        for p in (1, 2, 3, 9):
            pre(p)
        batch(0, [0, 1, 2, 3], 0)
        for p in (4, 5, 6, 7):
            pre(p)
        batch(0, [4, 5, 6, 7], 4)
        for p in (10, 11, 12, 13):
            pre(p)
        batch(0, [8], 8)
        batch(1, [8, 9, 10, 11], 9)
        for p in (14, 15):
            pre(p)
        batch(1, [12, 13, 14, 15], 13)

    nc.compile()
    return nc


def _get_compiled():
    if "nc" not in _COMPILED:
        _COMPILED["nc"] = _build_bass()
    return _COMPILED["nc"]


def host_side_inputs(x, attn):
    """Per-core input maps. w_sq/asq are tiny host-precomputed functions of
    attn_vectors; xt is a per-core rotated, transposed bf16 copy of x."""
    import ml_dtypes

    bf16 = ml_dtypes.bfloat16
    ab = attn.astype(bf16).astype(np.float32)
    w_sq = np.zeros((P, CHD * H), dtype=np.float32)
    for c in range(CHD):
        w_sq[:, c * H : (c + 1) * H] = (ab[:, c * P : (c + 1) * P] ** 2).T
    w_sq = w_sq.astype(bf16)
    asq = np.zeros((P, KCH), dtype=np.float32)
    for h in range(H):
        for c in range(CHD):
            asq[:, h * CHD + c] = BETA * attn[h, c * P : (c + 1) * P] ** 2
    xb = x.astype(bf16)
    ins = []
    for cid in range(NCORES):
        xr = np.roll(xb, -PANEL * cid, axis=0)  # [N, D] bf16
        xtc = np.ascontiguousarray(xr.T).reshape(CHD, P, N)
        ins.append({"xt": xtc, "w_sq": w_sq, "asq": asq})
    return ins


def _core_blocks(cid):
    """[(t, pi, pj)] global block positions for the 17 blocks of core cid."""
    blocks = []
    for t in range(NBLK):
        if t <= 8:
            pi, pj = cid, (cid + t) % NPAN
        else:
            pi, pj = cid + 8, (cid + 8 + (t - 9)) % NPAN
        blocks.append((t, pi, pj))
    return blocks


def assemble(results):
    out = np.empty((N, N), dtype=np.float32)
    for cid in range(NCORES):
        blks = np.asarray(results[cid]["out"]).astype(np.float32)
        for t, pi, pj in _core_blocks(cid):
            b = blks[t]
            ri = slice(pi * PANEL, (pi + 1) * PANEL)
            rj = slice(pj * PANEL, (pj + 1) * PANEL)
            if pi == pj:
                out[ri, rj] = 0.5 * (b + b.T)
            else:
                out[ri, rj] = b
                out[rj, ri] = b.T
    # rows are L2-normalized: diag(Y Y^T / H) == 1 exactly
    np.fill_diagonal(out, 1.0)
    return out


def kernel(**inputs) -> np.ndarray:
    from concourse import bass_utils

    x = np.ascontiguousarray(np.asarray(inputs["x"], dtype=np.float32))
    attn = np.ascontiguousarray(np.asarray(inputs["attn_vectors"], dtype=np.float32))
    nc = _get_compiled()
    res = bass_utils.run_bass_kernel_spmd(
        nc, host_side_inputs(x, attn), core_ids=list(range(NCORES))
    )
    return assemble(res.results)
